# revision 10
# baseline (speedup 1.0000x reference)
"""Locally-connected 3x3 block (LCBlock) Trainium2 kernel.

Computes out = ELU(einsum('ocdkij,bcdkij->boij', weights, unfold(x)))
for x:[16,32,64,64] f32, weights:[32,32,3,3,64,64] f32.

Strategy (8 NeuronCores, SPMD, no collectives):
  - Spatially shard H=64 into 8 strips of 8 rows; each core gets its strip's
    per-position weights (they shard perfectly) and a 10-row halo'd slab of x.
  - Per position p=(y,x) the LC contraction is a tiny matmul
    [B=16, CK=288] x [CK=288, O=32].  We run it on the PE as 3 PSUM-accumulated
    matmuls (one per dj kernel column): lhsT = patch [K=96=(3di x 32c), M=16b]
    (cheap LDWEIGHTS: cost scales with columns=16), rhs = weights
    [96, 32o] (the big tensor streams as the moving operand).  4 positions run
    concurrently in the 4 PE column-groups via tile_position.
  - bf16 operands (fp32 PSUM accumulation) halve the HBM roofline.
  - ELU = max(x, exp(min(x,0))-1): 2 DVE ops + 1 ACT op per row-wave.
Host side packs/scatters inputs and gathers the 8 output strips.
"""

import os
import sys

import numpy as np

for _p in ("/opt/trn_rl_repo", "/root/.axon_site/_ro/trn_rl_repo"):
    if os.path.isdir(_p) and _p not in sys.path:
        sys.path.insert(0, _p)

import ml_dtypes

import concourse.bacc as bacc
import concourse.mybir as mybir
import concourse.tile as tile
from concourse.bass_interp import get_hw_module
from concourse.bass_utils import run_bass_kernel_spmd

BF16 = ml_dtypes.bfloat16

# Problem shape (hardcoded per contract).
B, C, O, H, W = 16, 32, 32, 64, 64
NCORES = 8
HL = H // NCORES  # local rows per core
KW = 3  # conv kernel size
PART = KW * C  # 96 partitions: (di, c)
XW = W + 2  # padded row width
XFREE = HL * XW * B  # x slab free elems/partition
WCH = 4 * 16 * KW * O  # weight elems/partition per row-wave (j, pbl, dj, o)
WFREE = HL * WCH
OUTF = HL * 16 * O  # out free elems/partition: (w, pbl, o)

_CACHE = {}


def _build(hw=True, reps=1, variant="full", loop_n=None):
    nc = bacc.Bacc(
        "TRN2", target_bir_lowering=False, debug=False, num_devices=NCORES
    )
    xs_d = nc.dram_tensor("xs", [PART, XFREE], mybir.dt.bfloat16, kind="ExternalInput")
    w_d = nc.dram_tensor("w", [PART, WFREE], mybir.dt.bfloat16, kind="ExternalInput")
    out_d = nc.dram_tensor("out", [4, 16, OUTF], mybir.dt.float32, kind="ExternalOutput")

    with tile.TileContext(nc) as tc:
        with (
            tc.tile_pool(name="xp", bufs=1) as xp,
            tc.tile_pool(name="wp", bufs=3) as wp,
            tc.tile_pool(name="pp", bufs=3, space="PSUM") as pp,
            tc.tile_pool(name="op", bufs=1) as op,
            tc.tile_pool(name="tp", bufs=2) as tp,
        ):
          import contextlib

          loop_cm = tc.For_i(0, loop_n, 1) if loop_n else contextlib.nullcontext()
          with loop_cm:
           for _rep in range(reps):
            x_t = xp.tile([PART, XFREE], mybir.dt.bfloat16, tag="x")
            nc.sync.dma_start(x_t[:], xs_d[:])
            out_t = op.tile([128, OUTF], mybir.dt.float32, tag="o")

            for wv in range(HL):  # one image row per wave
                w_t = wp.tile([PART, WCH], mybir.dt.bfloat16, tag="w")
                nc.sync.dma_start(w_t[:], w_d[:][:, wv * WCH:(wv + 1) * WCH])
                ps = pp.tile([128, 512], mybir.dt.float32, tag="ps")
                # init rows the col-tiled matmuls never touch (ELU reads all 128)
                nc.vector.memset(ps[:], 0.0)
                if variant != "dma_only":
                    for pbl in range(16):
                        for j in range(4):
                            roff = (j * 16 + pbl) * 3 * O
                            for dj in range(KW):
                                lo = (wv * XW + pbl * 4 + j + dj) * B
                                nc.tensor.matmul(
                                    ps[32 * j:32 * j + B, pbl * 32:(pbl + 1) * 32],
                                    x_t[:, lo:lo + B],
                                    w_t[:, roff + dj * O:roff + (dj + 1) * O],
                                    start=(dj == 0),
                                    stop=(dj == KW - 1),
                                    tile_position=(0, 32 * j),
                                )
                if variant in ("full",):
                    # ELU: out = max(psum, exp(min(psum, 0)) - 1)
                    t1 = tp.tile([128, 512], mybir.dt.float32, tag="t1")
                    nc.vector.tensor_scalar_min(t1[:], ps[:], 0.0)
                    nc.scalar.activation(
                        t1[:], t1[:], mybir.ActivationFunctionType.Exp
                    )
                    nc.vector.scalar_tensor_tensor(
                        out_t[:, wv * 512:(wv + 1) * 512],
                        t1[:],
                        -1.0,
                        ps[:],
                        op0=mybir.AluOpType.add,
                        op1=mybir.AluOpType.max,
                    )
                else:
                    # cheap evacuation so deps/out exist: copy psum -> out
                    nc.vector.tensor_copy(
                        out_t[:, wv * 512:(wv + 1) * 512], ps[:]
                    )
            oap = out_d.ap()
            for j in range(4):
                nc.sync.dma_start(oap[j], out_t[32 * j:32 * j + 16, :])

    nc.compile()
    if hw:
        nc.m = get_hw_module(nc.m)
    return nc


def _pack_inputs(x, weights):
    """Host-side scatter: per-core bf16 slabs."""
    xpad = np.pad(x, ((0, 0), (0, 0), (1, 1), (1, 1))).astype(BF16)  # [B,C,66,66]
    wb = np.asarray(weights).astype(BF16)  # [O,C,3,3,H,W]
    in_maps = []
    for k in range(NCORES):
        # x slab: [di*32+c, y, xx, b] = xpad[b, c, 8k+y+di, xx]
        slabs = [
            np.transpose(xpad[:, :, 8 * k + di:8 * k + di + HL, :], (1, 2, 3, 0))
            for di in range(KW)
        ]
        xs_k = np.ascontiguousarray(np.stack(slabs, 0)).reshape(PART, XFREE)
        # weights: [di*32+c, w, j, pbl, dj, o] = W[o, c, di, dj, 8k+w, pbl*4+j]
        wc = wb[:, :, :, :, 8 * k:8 * (k + 1), :].reshape(O, C, KW, KW, HL, 16, 4)
        w_k = np.ascontiguousarray(
            np.transpose(wc, (2, 1, 4, 6, 5, 3, 0))
        ).reshape(PART, WFREE)
        in_maps.append({"xs": xs_k, "w": w_k})
    return in_maps


def _unpack_outputs(results):
    out = np.empty((B, O, H, W), dtype=np.float32)
    for k in range(NCORES):
        arr = results[k]["out"].reshape(4, 16, HL, 16, O)  # [j, b, w, pbl, o]
        strip = np.transpose(arr, (1, 4, 2, 3, 0)).reshape(B, O, HL, W)
        out[:, :, 8 * k:8 * (k + 1), :] = strip
    return out


def run(x, weights, trace=False):
    if "nc" not in _CACHE:
        _CACHE["nc"] = _build()
    nc = _CACHE["nc"]
    in_maps = _pack_inputs(np.asarray(x), np.asarray(weights))
    res = run_bass_kernel_spmd(nc, in_maps, list(range(NCORES)), trace=trace)
    return _unpack_outputs(res.results), res


def kernel(x, weights):
    out, _ = run(x, weights)
    return out


# revision 13
# speedup vs baseline: 1.4550x; 1.4550x over previous
"""Locally-connected 3x3 block (LCBlock) Trainium2 kernel.

Computes out = ELU(einsum('ocdkij,bcdkij->boij', weights, unfold(x)))
for x:[16,32,64,64] f32, weights:[32,32,3,3,64,64] f32.

Strategy (8 NeuronCores, SPMD, no collectives):
  - Spatially shard H=64 into 8 strips of 8 rows; each core gets its strip's
    per-position weights (they shard perfectly) and a 10-row halo'd slab of x.
  - Per position p=(y,x) the LC contraction is a tiny matmul
    [B=16, CK=288] x [CK=288, O=32].  We run it on the PE as 3 PSUM-accumulated
    matmuls (one per dj kernel column): lhsT = patch [K=96=(3di x 32c), M=16b]
    (cheap LDWEIGHTS: cost scales with columns=16), rhs = weights
    [96, 32o] (the big tensor streams as the moving operand).  4 positions run
    concurrently in the 4 PE column-groups via tile_position.
  - bf16 operands (fp32 PSUM accumulation) halve the HBM roofline.
  - ELU = max(x, exp(min(x,0))-1): 2 DVE ops + 1 ACT op per row-wave.
Host side packs/scatters inputs and gathers the 8 output strips.
"""

import os
import sys

import numpy as np

for _p in ("/opt/trn_rl_repo", "/root/.axon_site/_ro/trn_rl_repo"):
    if os.path.isdir(_p) and _p not in sys.path:
        sys.path.insert(0, _p)

import ml_dtypes

import concourse.bacc as bacc
import concourse.mybir as mybir
import concourse.tile as tile
from concourse.bass_interp import get_hw_module
from concourse.bass_utils import run_bass_kernel_spmd

BF16 = ml_dtypes.bfloat16

# Problem shape (hardcoded per contract).
B, C, O, H, W = 16, 32, 32, 64, 64
NCORES = 8
HL = H // NCORES  # local rows per core
KW = 3  # conv kernel size
PART = KW * C  # 96 partitions: (di, c)
XW = W + 2  # padded row width
XFREE = HL * XW * B  # x slab free elems/partition
WCH = 4 * 16 * KW * O  # weight elems/partition per row-wave (j, pbl, dj, o)
WFREE = HL * WCH
OUTF = HL * 16 * O  # out free elems/partition: (w, pbl, o)

_CACHE = {}


def _build(hw=True, reps=1, variant="full", loop_n=None):
    nc = bacc.Bacc(
        "TRN2", target_bir_lowering=False, debug=False, num_devices=NCORES
    )
    xs_d = nc.dram_tensor("xs", [PART, XFREE], mybir.dt.bfloat16, kind="ExternalInput")
    w_d = nc.dram_tensor("w", [PART, WFREE], mybir.dt.bfloat16, kind="ExternalInput")
    out_d = nc.dram_tensor("out", [4, 16, OUTF], mybir.dt.float32, kind="ExternalOutput")

    with tile.TileContext(nc) as tc:
        with (
            tc.tile_pool(name="xp", bufs=1) as xp,
            tc.tile_pool(name="wp", bufs=3) as wp,
            tc.tile_pool(name="pp", bufs=3, space="PSUM") as pp,
            tc.tile_pool(name="op", bufs=1) as op,
            tc.tile_pool(name="tp", bufs=2) as tp,
        ):
          import contextlib

          loop_cm = tc.For_i(0, loop_n, 1) if loop_n else contextlib.nullcontext()
          with loop_cm:
           for _rep in range(reps):
            x_t = xp.tile([PART, XFREE], mybir.dt.bfloat16, tag="x")
            nc.sync.dma_start(x_t[:], xs_d[:])
            out_t = op.tile([128, OUTF], mybir.dt.float32, tag="o")

            for wv in range(HL):  # one image row per wave
                w_t = wp.tile([PART, WCH], mybir.dt.bfloat16, tag="w")
                nc.sync.dma_start(w_t[:], w_d[:][:, wv * WCH:(wv + 1) * WCH])
                ps = pp.tile([128, 512], mybir.dt.float32, tag="ps")
                # zero-fill: matmuls pure-accumulate (start=False) onto this,
                # and ELU reads rows the col-tiled matmuls never touch
                nc.vector.memset(ps[:], 0.0)
                if variant != "dma_only":
                    # one MM per (xx, j): patch col xx serves dj=0,1,2 for
                    # positions x = xx, xx-1, xx-2 (adjacent PSUM slots)
                    coff = 0
                    for xx in range(18):
                        x_lo, x_hi = max(0, xx - 2), min(15, xx)
                        n = x_hi - x_lo + 1
                        for j in range(4):
                            lo = (wv * XW + 16 * j + xx) * B
                            nc.tensor.matmul(
                                ps[32 * j:32 * j + B,
                                   32 * x_lo:32 * (x_hi + 1)],
                                x_t[:, lo:lo + B],
                                w_t[:, coff + j * n * O:coff + (j + 1) * n * O],
                                start=False,
                                stop=True,
                                skip_group_check=True,
                                tile_position=(0, 32 * j),
                            )
                        coff += 4 * n * O
                if variant in ("full",):
                    # ELU: out = max(psum, exp(min(psum, 0)) - 1)
                    t1 = tp.tile([128, 512], mybir.dt.float32, tag="t1")
                    nc.vector.tensor_scalar_min(t1[:], ps[:], 0.0)
                    nc.scalar.activation(
                        t1[:], t1[:], mybir.ActivationFunctionType.Exp
                    )
                    nc.vector.scalar_tensor_tensor(
                        out_t[:, wv * 512:(wv + 1) * 512],
                        t1[:],
                        -1.0,
                        ps[:],
                        op0=mybir.AluOpType.add,
                        op1=mybir.AluOpType.max,
                    )
                else:
                    # cheap evacuation so deps/out exist: copy psum -> out
                    nc.vector.tensor_copy(
                        out_t[:, wv * 512:(wv + 1) * 512], ps[:]
                    )
            oap = out_d.ap()
            for j in range(4):
                nc.sync.dma_start(oap[j], out_t[32 * j:32 * j + 16, :])

    nc.compile()
    if hw:
        nc.m = get_hw_module(nc.m)
    return nc


def _pack_inputs(x, weights):
    """Host-side scatter: per-core bf16 slabs."""
    xpad = np.pad(x, ((0, 0), (0, 0), (1, 1), (1, 1))).astype(BF16)  # [B,C,66,66]
    wb = np.asarray(weights).astype(BF16)  # [O,C,3,3,H,W]
    in_maps = []
    for k in range(NCORES):
        # x slab: [di*32+c, y, xx, b] = xpad[b, c, 8k+y+di, xx]
        slabs = [
            np.transpose(xpad[:, :, 8 * k + di:8 * k + di + HL, :], (1, 2, 3, 0))
            for di in range(KW)
        ]
        xs_k = np.ascontiguousarray(np.stack(slabs, 0)).reshape(PART, XFREE)
        # weights, merged-xx layout: per (y, xx, j), 32-col blocks for
        # x = x_lo..x_hi ascending (dj = xx-x descending):
        #   block = W[o, c, di, dj, 8k+y, 16j+x] as [di*32+c, y, o]
        wc = np.transpose(
            wb[:, :, :, :, 8 * k:8 * (k + 1), :], (2, 1, 3, 4, 5, 0)
        )  # [di, c, dj, y, x, o]
        wc = wc.reshape(PART, KW, HL, W, O)  # [(di,c), dj, y, x, o]
        w_k = np.empty((PART, HL, WCH), dtype=BF16)
        coff = 0
        for xx in range(18):
            x_lo, x_hi = max(0, xx - 2), min(15, xx)
            n = x_hi - x_lo + 1
            for j in range(4):
                for t, xr in enumerate(range(x_lo, x_hi + 1)):
                    dj = xx - xr
                    c0 = coff + j * n * O + t * O
                    # [(di,c), y, o]
                    w_k[:, :, c0:c0 + O] = wc[:, dj, :, 16 * j + xr, :]
            coff += 4 * n * O
        in_maps.append({"xs": xs_k, "w": w_k.reshape(PART, WFREE)})
    return in_maps


def _unpack_outputs(results):
    out = np.empty((B, O, H, W), dtype=np.float32)
    for k in range(NCORES):
        arr = results[k]["out"].reshape(4, 16, HL, 16, O)  # [j, b, w, slot, o]
        strip = np.transpose(arr, (1, 4, 2, 0, 3)).reshape(B, O, HL, W)
        out[:, :, 8 * k:8 * (k + 1), :] = strip
    return out


def run(x, weights, trace=False):
    if "nc" not in _CACHE:
        _CACHE["nc"] = _build()
    nc = _CACHE["nc"]
    in_maps = _pack_inputs(np.asarray(x), np.asarray(weights))
    res = run_bass_kernel_spmd(nc, in_maps, list(range(NCORES)), trace=trace)
    return _unpack_outputs(res.results), res


def kernel(x, weights):
    out, _ = run(x, weights)
    return out


# revision 18
# speedup vs baseline: 1.6015x; 1.1006x over previous
"""Locally-connected 3x3 block (LCBlock) Trainium2 kernel.

Computes out = ELU(einsum('ocdkij,bcdkij->boij', weights, unfold(x)))
for x:[16,32,64,64] f32, weights:[32,32,3,3,64,64] f32.

Strategy (8 NeuronCores, SPMD, no collectives):
  - Spatially shard H=64 into 8 strips of 8 rows; each core gets its strip's
    per-position weights (they shard perfectly) and a 10-row halo'd slab of x.
  - Per position p=(y,x) the LC contraction is a tiny matmul
    [B=16, CK=288] x [CK=288, O=32].  We run it on the PE as 3 PSUM-accumulated
    matmuls (one per dj kernel column): lhsT = patch [K=96=(3di x 32c), M=16b]
    (cheap LDWEIGHTS: cost scales with columns=16), rhs = weights
    [96, 32o] (the big tensor streams as the moving operand).  4 positions run
    concurrently in the 4 PE column-groups via tile_position.
  - bf16 operands (fp32 PSUM accumulation) halve the HBM roofline.
  - ELU = max(x, exp(min(x,0))-1): 2 DVE ops + 1 ACT op per row-wave.
Host side packs/scatters inputs and gathers the 8 output strips.
"""

import os
import sys

import numpy as np

for _p in ("/opt/trn_rl_repo", "/root/.axon_site/_ro/trn_rl_repo"):
    if os.path.isdir(_p) and _p not in sys.path:
        sys.path.insert(0, _p)

import ml_dtypes

import concourse.bacc as bacc
import concourse.mybir as mybir
import concourse.tile as tile
from concourse.bass_interp import get_hw_module
from concourse.bass_utils import run_bass_kernel_spmd

BF16 = ml_dtypes.bfloat16

# Problem shape (hardcoded per contract).
B, C, O, H, W = 16, 32, 32, 64, 64
NCORES = 8
HL = H // NCORES  # local rows per core
KW = 3  # conv kernel size
PART = KW * C  # 96 partitions: (di, c)
XW = W + 2  # padded row width
XFREE = HL * XW * B  # x slab free elems/partition
WCH = 4 * 16 * KW * O  # weight elems/partition per row-wave (j, pbl, dj, o)
WFREE = HL * WCH
OUTF = HL * 16 * O  # out free elems/partition: (w, pbl, o)

_CACHE = {}


def _build(hw=True, reps=1, variant="full", loop_n=None, rpw=1, wbufs=None):
    nc = bacc.Bacc(
        "TRN2", target_bir_lowering=False, debug=False, num_devices=NCORES
    )
    xs_d = nc.dram_tensor("xs", [PART, XFREE], mybir.dt.bfloat16, kind="ExternalInput")
    w_d = nc.dram_tensor("w", [PART, WFREE], mybir.dt.bfloat16, kind="ExternalInput")
    out_d = nc.dram_tensor("out", [4, 16, OUTF], mybir.dt.float32, kind="ExternalOutput")

    if wbufs is None:
        wbufs = {1: 3, 2: 3, 4: 2, 8: 1}[rpw]
    with tile.TileContext(nc) as tc:
        with (
            tc.tile_pool(name="xp", bufs=1) as xp,
            tc.tile_pool(name="wp", bufs=wbufs) as wp,
            tc.tile_pool(name="pp", bufs=3, space="PSUM") as pp,
            tc.tile_pool(name="op", bufs=1) as op,
            tc.tile_pool(name="tp", bufs=2) as tp,
        ):
          import contextlib

          loop_cm = tc.For_i(0, loop_n, 1) if loop_n else contextlib.nullcontext()
          with loop_cm:
           for _rep in range(reps):
            x_t = xp.tile([PART, XFREE], mybir.dt.bfloat16, tag="x")
            nc.sync.dma_start(x_t[:], xs_d[:])
            out_t = op.tile([128, OUTF], mybir.dt.float32, tag="o")

            for wg in range(HL // rpw):  # rpw image rows per DMA chunk
              w_t = wp.tile([PART, rpw * WCH], mybir.dt.bfloat16, tag="w")
              nc.sync.dma_start(
                  w_t[:], w_d[:][:, wg * rpw * WCH:(wg + 1) * rpw * WCH]
              )
              for r in range(rpw):
                wv = wg * rpw + r
                ps = pp.tile([128, 512], mybir.dt.float32, tag="ps")
                # zero-fill: matmuls pure-accumulate (start=False) onto this,
                # and ELU reads rows the col-tiled matmuls never touch
                nc.vector.memset(ps[:], 0.0)
                if variant != "dma_only":
                    # one MM per (xx, j): patch col xx serves dj=0,1,2 for
                    # positions x = xx, xx-1, xx-2 (adjacent PSUM slots)
                    coff = 0
                    for xx in range(18):
                        x_lo, x_hi = max(0, xx - 2), min(15, xx)
                        n = x_hi - x_lo + 1
                        for j in range(4):
                            lo = (wv * XW + 16 * j + xx) * B
                            nc.tensor.matmul(
                                ps[32 * j:32 * j + B,
                                   32 * x_lo:32 * (x_hi + 1)],
                                x_t[:, lo:lo + B],
                                w_t[:, r * WCH + coff + j * n * O:
                                     r * WCH + coff + (j + 1) * n * O],
                                start=False,
                                stop=True,
                                skip_group_check=True,
                                tile_position=(0, 32 * j),
                            )
                        coff += 4 * n * O
                if variant in ("full",):
                    # ELU: out = max(psum, exp(min(psum, 0)) - 1)
                    t1 = tp.tile([128, 512], mybir.dt.float32, tag="t1")
                    nc.vector.tensor_scalar_min(t1[:], ps[:], 0.0)
                    nc.scalar.activation(
                        t1[:], t1[:], mybir.ActivationFunctionType.Exp
                    )
                    nc.vector.scalar_tensor_tensor(
                        out_t[:, wv * 512:(wv + 1) * 512],
                        t1[:],
                        -1.0,
                        ps[:],
                        op0=mybir.AluOpType.add,
                        op1=mybir.AluOpType.max,
                    )
                else:
                    # cheap evacuation so deps/out exist: copy psum -> out
                    nc.vector.tensor_copy(
                        out_t[:, wv * 512:(wv + 1) * 512], ps[:]
                    )
            oap = out_d.ap()
            for j in range(4):
                nc.sync.dma_start(oap[j], out_t[32 * j:32 * j + 16, :])

    nc.compile()
    if hw:
        nc.m = get_hw_module(nc.m)
    return nc


def _pack_inputs(x, weights):
    """Host-side scatter: per-core bf16 slabs."""
    xpad = np.pad(x, ((0, 0), (0, 0), (1, 1), (1, 1))).astype(BF16)  # [B,C,66,66]
    wb = np.asarray(weights).astype(BF16)  # [O,C,3,3,H,W]
    in_maps = []
    for k in range(NCORES):
        # x slab: [di*32+c, y, xx, b] = xpad[b, c, 8k+y+di, xx]
        slabs = [
            np.transpose(xpad[:, :, 8 * k + di:8 * k + di + HL, :], (1, 2, 3, 0))
            for di in range(KW)
        ]
        xs_k = np.ascontiguousarray(np.stack(slabs, 0)).reshape(PART, XFREE)
        # weights, merged-xx layout: per (y, xx, j), 32-col blocks for
        # x = x_lo..x_hi ascending (dj = xx-x descending):
        #   block = W[o, c, di, dj, 8k+y, 16j+x] as [di*32+c, y, o]
        wc = np.transpose(
            wb[:, :, :, :, 8 * k:8 * (k + 1), :], (2, 1, 3, 4, 5, 0)
        )  # [di, c, dj, y, x, o]
        wc = wc.reshape(PART, KW, HL, W, O)  # [(di,c), dj, y, x, o]
        w_k = np.empty((PART, HL, WCH), dtype=BF16)
        coff = 0
        for xx in range(18):
            x_lo, x_hi = max(0, xx - 2), min(15, xx)
            n = x_hi - x_lo + 1
            for j in range(4):
                for t, xr in enumerate(range(x_lo, x_hi + 1)):
                    dj = xx - xr
                    c0 = coff + j * n * O + t * O
                    # [(di,c), y, o]
                    w_k[:, :, c0:c0 + O] = wc[:, dj, :, 16 * j + xr, :]
            coff += 4 * n * O
        in_maps.append({"xs": xs_k, "w": w_k.reshape(PART, WFREE)})
    return in_maps


def _unpack_outputs(results):
    out = np.empty((B, O, H, W), dtype=np.float32)
    for k in range(NCORES):
        arr = results[k]["out"].reshape(4, 16, HL, 16, O)  # [j, b, w, slot, o]
        strip = np.transpose(arr, (1, 4, 2, 0, 3)).reshape(B, O, HL, W)
        out[:, :, 8 * k:8 * (k + 1), :] = strip
    return out


def run(x, weights, trace=False):
    if "nc" not in _CACHE:
        _CACHE["nc"] = _build()
    nc = _CACHE["nc"]
    in_maps = _pack_inputs(np.asarray(x), np.asarray(weights))
    res = run_bass_kernel_spmd(nc, in_maps, list(range(NCORES)), trace=trace)
    return _unpack_outputs(res.results), res


def kernel(x, weights):
    out, _ = run(x, weights)
    return out
